# revision 1
# baseline (speedup 1.0000x reference)
"""Local (causal) attention block on 8 TRN2 NeuronCores.

Reference computation (B=2, T=2048, C=1024, H=16, D=64):
    q,k,v = x@Wq.T, x@Wk.T, x@Wv.T          (per-head D=64)
    att   = softmax(causal_mask(q k^T / sqrt(D)))
    out   = (att v) @ Wo.T
(The reference's "window" band mask reduces exactly to the plain strict
causal mask, so this is full causal attention.)

Sharding (SPMD-uniform across the 8 cores):
  core c: batch b = c//4, head-group g = c%4 (heads 4g..4g+3),
  output-channel group g (columns 256g..256g+255).

Structure (single pipelined emission, no phase barriers; the Tile
scheduler weaves projection matmuls into scalar-bound attention gaps):
  - Host casts x^T and all weights to bf16 (halves input DMA, removes
    on-chip casts); x DMA'd in ci-pair pieces so the first projection
    chain starts after ~128KB; warmup matmuls + exp spin up the PE HAM
    clock and the ACT table during the input DMAs.
  - Projection chain order (Q/K co-group 0 first) unblocks each
    chunk's attention pass A as early as possible.
  - Attention per chunk: two passes (one head pair each).  The pair's
    S^T scores ([128,2,512] f32 psum, double-buffered, kv on
    partitions / q on free axis) batch into ONE exp per kv tile
    ([128,1024]) so ScalarE runs gap-free; S(k+1) overlaps exp(k).
    Diagonal kv tiles get a pre-exp additive -1e5 triangle and a
    shrunken valid q range (causal block savings on S/exp/AV).
  - Rowsums via a ones-column appended to V (psum row D);
    reciprocal_approx_fast + gpsimd partition-broadcast + multiply
    normalizes O^T per pass.
  - Each pass's O^T co-group exchanged immediately (8 small DRAM-bounce
    AllGathers) so collectives pipeline with compute; chunk order
    0,1,3,2 gives the big chunk 3 projection weave material and hides
    its exchanges under chunk 2; chunk 2's final exchange is q-halved.
  - Output projection oc-sharded, trailing each exchange by at least a
    chunk of attention so static engine orders never head-of-line
    block on an in-flight AllGather.
PSUM budget (8 banks): S pair tile x2 bufs = 4, two AV accumulators
(ones-column rowsums) = 2, projection/out-proj chain psum x2 = 2.
"""

import sys

for _p in ("/opt/trn_rl_repo",):
    if _p not in sys.path:
        sys.path.append(_p)

import ml_dtypes
import numpy as np

import concourse.bass as bass
import concourse.mybir as mybir
import concourse.tile as tile
from concourse import bacc
from concourse.bass import ts
from concourse.bass_utils import run_bass_kernel_spmd

B, T, C = 2, 2048, 1024
H, D = 16, 64
SCALE = 1.0 / np.sqrt(D)
N_CORES = 8
HPC = H // 4          # heads per core = 4
COC = C // 4          # channels per core = 256
F32 = mybir.dt.float32
BF16 = mybir.dt.bfloat16
NEG = -1.0e5          # additive mask value (pre-scale)
NQC = T // 512        # 4 q-chunks / t-chunks of 512
NKT = T // 128        # 16 kv tiles of 128
NCT = C // 128        # 8 contraction tiles


def build_nc():
    nc = bacc.Bacc(
        "TRN2",
        target_bir_lowering=False,
        debug=False,
        num_devices=N_CORES,
    )
    xT_d = nc.dram_tensor("xT", [C, T], BF16, kind="ExternalInput").ap()
    wqT_d = nc.dram_tensor("wqT", [C, COC], BF16, kind="ExternalInput").ap()
    wkT_d = nc.dram_tensor("wkT", [C, COC], BF16, kind="ExternalInput").ap()
    wvT_d = nc.dram_tensor("wvT", [C, COC], BF16, kind="ExternalInput").ap()
    woT_d = nc.dram_tensor("woT", [C, COC], BF16, kind="ExternalInput").ap()
    out_d = nc.dram_tensor("out", [T, COC], F32, kind="ExternalOutput").ap()

    xT_r = xT_d.rearrange("(a p) t -> p a t", p=128)

    with tile.TileContext(nc) as tc:
        with (
            tc.tile_pool(name="main", bufs=1) as main,
            tc.tile_pool(name="xp", bufs=2) as xp,
            tc.tile_pool(name="work", bufs=4) as work,
            tc.tile_pool(name="psum", bufs=1, space="PSUM") as psum,
            tc.tile_pool(name="dram", bufs=2, space="DRAM") as dram,
        ):
            # ---- long-lived SBUF tensors ----
            qT_sb = main.tile([128, 2, T], BF16)         # [co 256, t]
            kT_sb = main.tile([128, 2, T], BF16)
            v_sb = main.tile([128, NKT, HPC, D + 1], BF16)  # V + ones col
            wq_sb = main.tile([128, NCT, COC], BF16)
            wk_sb = main.tile([128, NCT, COC], BF16)
            wv_sb = main.tile([128, NCT, COC], BF16)
            wo_sb = main.tile([128, NCT, COC], BF16)
            otall = [
                main.tile([128, 2, 512], BF16, name=f"otall{j}") for j in range(NQC)
            ]
            otfull = [
                main.tile([128, NCT, 512], BF16, name=f"otfull{j}")
                for j in range(NQC)
            ]

            # ---- input DMAs: x in ci-pair pieces so the first projection
            # chain starts after ~128KB, weights on other queues ----
            def dma_x(j, xt):
                for piece in range(4):
                    nc.sync.dma_start(
                        out=xt[:, 2 * piece : 2 * piece + 2, :],
                        in_=xT_r[:, 2 * piece : 2 * piece + 2, ts(j, 512)],
                    )

            xj_t = []
            x0 = xp.tile([128, NCT, 512], BF16, tag="x", name="x0")
            dma_x(0, x0)
            xj_t.append(x0)
            nc.scalar.dma_start(
                out=wq_sb[:], in_=wqT_d.rearrange("(a p) t -> p a t", p=128)
            )
            nc.scalar.dma_start(
                out=wk_sb[:], in_=wkT_d.rearrange("(a p) t -> p a t", p=128)
            )
            nc.gpsimd.dma_start(
                out=wv_sb[:], in_=wvT_d.rearrange("(a p) t -> p a t", p=128)
            )
            nc.gpsimd.dma_start(
                out=wo_sb[:], in_=woT_d.rearrange("(a p) t -> p a t", p=128)
            )

            # ---- warmups during the input DMAs: ACT table load + PE HAM
            # spin-up ----
            wu = main.tile([128, 512], BF16)
            nc.vector.memset(wu[:], 0.0)
            wud = main.tile([1, 8], BF16)
            nc.scalar.activation(
                wud[:], wu[0:1, 0:8], mybir.ActivationFunctionType.Exp
            )
            wups = psum.tile([128, 512], F32, tag="pp", bufs=2, name="wups")
            for _ in range(9):
                nc.tensor.matmul(
                    wups[:], wu[:, 0:128], wu[:], start=True, stop=True
                )

            # additive causal triangle for diagonal blocks, replicated for
            # a head pair: tri[k, i, q] = 0 if q >= k else -1e5
            tri32 = main.tile([128, 2, 128], F32)
            nc.gpsimd.memset(tri32[:], 0.0)
            nc.gpsimd.affine_select(
                out=tri32[:],
                in_=tri32[:],
                pattern=[[0, 2], [1, 128]],
                compare_op=mybir.AluOpType.is_ge,
                fill=NEG,
                base=0,
                channel_multiplier=-1,
            )
            tri = main.tile([128, 2, 128], BF16)
            nc.vector.tensor_copy(tri[:], tri32[:])

            def proj_stage(j):
                """QKV projections for t-chunk j.  Chain order puts the
                attention-critical pieces first: pass A of attn(j) needs
                only qT/kT co-group 0 (plus V for the diagonal tiles)."""
                xj = xj_t[j]

                def qk_chain(w_sb, dst, co):
                    ps = psum.tile([128, 512], F32, tag="pp", bufs=2, name="psA")
                    for ci in range(NCT):
                        nc.tensor.matmul(
                            ps[:],
                            w_sb[:, ci, ts(co, 128)],
                            xj[:, ci, :],
                            start=(ci == 0),
                            stop=(ci == NCT - 1),
                        )
                    nc.vector.tensor_copy(dst[:, co, ts(j, 512)], ps[:])

                def v_chain(tl):
                    tt = 4 * j + tl
                    ps = psum.tile([128, COC], F32, tag="pp", bufs=2, name="psB")
                    for ci in range(NCT):
                        nc.tensor.matmul(
                            ps[:],
                            xj[:, ci, ts(tl, 128)],
                            wv_sb[:, ci, :],
                            start=(ci == 0),
                            stop=(ci == NCT - 1),
                        )
                    nc.vector.tensor_copy(
                        v_sb[:, tt, :, 0:D],
                        ps[:].rearrange("p (h d) -> p h d", h=HPC),
                    )
                    nc.vector.memset(v_sb[:, tt, :, D], 1.0)

                qk_chain(wq_sb, qT_sb, 0)
                qk_chain(wk_sb, kT_sb, 0)
                for tl in range(4):
                    v_chain(tl)
                qk_chain(wq_sb, qT_sb, 1)
                qk_chain(wk_sb, kT_sb, 1)

            def attn_chunk(j, last=False):
                """Causal attention for q-chunk j: two passes, one head
                pair per pass.  The pair's S tile ([128,2,512] f32,
                double-buffered) batches the pair's exp into one ACTIVATE
                so ScalarE runs gap-free; S(k+1) overlaps exp(k)."""
                nkv = 4 * (j + 1)
                for pair in range(2):
                    h0, h1 = 2 * pair, 2 * pair + 1
                    o_ps = {
                        h: psum.tile(
                            [D + 1, 512], F32, tag=f"o{i}", bufs=1, name=f"o{i}"
                        )
                        for i, h in ((0, h0), (1, h1))
                    }
                    for k in range(nkv):
                        m = k - 4 * j          # >=0: diagonal block index
                        qs = 128 * m if m > 0 else 0
                        sT = psum.tile(
                            [128, 2, 512], F32, tag="s", bufs=2, name="s"
                        )
                        for i, h in ((0, h0), (1, h1)):
                            bp = 64 * (h % 2)
                            nc.tensor.matmul(
                                sT[:, i, qs:512],
                                kT_sb[bp : bp + 64, h // 2, ts(k, 128)],
                                qT_sb[bp : bp + 64, h // 2, 512 * j + qs : 512 * (j + 1)],
                                start=True,
                                stop=True,
                                tile_position=(bp, 0),
                                skip_group_check=True,
                            )
                        if m >= 0:  # diagonal: additive -1e5 triangle
                            nc.vector.tensor_add(
                                sT[:, :, qs : qs + 128],
                                sT[:, :, qs : qs + 128],
                                tri[:],
                            )
                        pt = work.tile([128, 2, 512], BF16, tag="pt")
                        nc.scalar.activation(
                            pt[:, :, qs:512],
                            sT[:, :, qs:512],
                            mybir.ActivationFunctionType.Exp,
                            scale=float(SCALE),
                        )
                        for i, h in ((0, h0), (1, h1)):
                            nc.tensor.matmul(
                                o_ps[h][:, qs:512],
                                v_sb[:, k, h, :],
                                pt[:, i, qs:512],
                                start=(k == 0),
                                stop=(k == nkv - 1),
                                skip_group_check=True,
                            )
                    # normalize: 1/rowsum (psum row D) partition-broadcast;
                    # ops grouped by engine so the two heads pipeline
                    stg, stgr, bcast = {}, {}, {}
                    for h in (h0, h1):
                        stg[h] = work.tile([1, 512], F32, tag="stg", name="stg")
                        nc.vector.tensor_copy(stg[h][:], o_ps[h][D : D + 1, :])
                    for h in (h0, h1):
                        stgr[h] = work.tile([1, 512], F32, tag="stgr", name="stgr")
                        nc.vector.reciprocal_approx_fast(
                            out=stgr[h][:], in_=stg[h][:]
                        )
                        bcast[h] = work.tile([64, 512], F32, tag="bcast", name="bcast")
                        nc.gpsimd.partition_broadcast(bcast[h][:], stgr[h][:])
                    for h in (h0, h1):
                        nc.vector.tensor_mul(
                            otall[j][64 * (h % 2) : 64 * (h % 2) + 64, h // 2, :],
                            o_ps[h][0:D, :],
                            bcast[h][:],
                        )
                    if last and pair == 1:
                        exchange_pass_half(j, pair, 0)
                        exchange_pass_half(j, pair, 1)
                    else:
                        exchange_pass(j, pair)

            def exchange_pass(j, pair):
                """AllGather one head-pair's O^T (co-group `pair`) across
                the batch's 4 cores, right after that pass's norm.  Core
                g's group lands at otfull rows 256g+128*pair."""
                bi = dram.tile([128, 512], BF16, tag="bi", name=f"bi{j}p{pair}")
                nc.sync.dma_start(out=bi[:], in_=otall[j][:, pair, :])
                bo = dram.tile([512, 512], BF16, tag="bo", name=f"bo{j}p{pair}")
                nc.gpsimd.collective_compute(
                    "AllGather",
                    mybir.AluOpType.bypass,
                    replica_groups=[[0, 1, 2, 3], [4, 5, 6, 7]],
                    ins=[bi.opt()],
                    outs=[bo.opt()],
                )
                nc.sync.dma_start(
                    out=otfull[j][:, pair::2, :],
                    in_=bo[:].rearrange("(r p) t -> p r t", p=128),
                )

            def exchange_pass_half(j, pair, half):
                """q-split exchange for the final pass: lets the last
                out-projections start as soon as their half arrives."""
                qs = 256 * half
                bi = dram.tile(
                    [128, 256], BF16, tag="bih", name=f"bi{j}p{pair}h{half}"
                )
                nc.sync.dma_start(out=bi[:], in_=otall[j][:, pair, qs : qs + 256])
                bo = dram.tile(
                    [512, 256], BF16, tag="boh", name=f"bo{j}p{pair}h{half}"
                )
                nc.gpsimd.collective_compute(
                    "AllGather",
                    mybir.AluOpType.bypass,
                    replica_groups=[[0, 1, 2, 3], [4, 5, 6, 7]],
                    ins=[bi.opt()],
                    outs=[bo.opt()],
                )
                nc.sync.dma_start(
                    out=otfull[j][:, pair::2, qs : qs + 256],
                    in_=bo[:].rearrange("(r p) t -> p r t", p=128),
                )

            def out_proj_q(j, ql):
                """oc-sharded output projection for one 128-row q-tile."""
                ps = psum.tile([128, COC], F32, tag="pp", bufs=2, name="po")
                for ci in range(NCT):
                    nc.tensor.matmul(
                        ps[:],
                        otfull[j][:, ci, ts(ql, 128)],
                        wo_sb[:, ci, :],
                        start=(ci == 0),
                        stop=(ci == NCT - 1),
                    )
                ot = work.tile([128, COC], F32, tag="outst")
                nc.vector.tensor_copy(ot[:], ps[:])
                nc.sync.dma_start(out=out_d[ts(4 * j + ql, 128), :], in_=ot[:])

            # ---- pipelined stages.  Chunk order 1,2,3,0: the tiny chunk
            # 0 runs last so only its (halved) final exchange is
            # tail-exposed; later-emitted projections weave into each
            # scalar-bound attention chunk by readiness.  Out-proj trails
            # its exchange by >=1 chunk of attention so static engine
            # orders never block on an in-flight AllGather. ----
            def dma_x_next(jn):
                xn = xp.tile([128, NCT, 512], BF16, tag="x", name=f"x{jn}")
                dma_x(jn, xn)
                xj_t.append(xn)

            dma_x_next(1)
            proj_stage(0)
            attn_chunk(0)
            dma_x_next(2)
            proj_stage(1)
            attn_chunk(1)
            dma_x_next(3)
            proj_stage(2)
            proj_stage(3)
            attn_chunk(3)
            for ql in range(4):
                out_proj_q(0, ql)
            attn_chunk(2, last=True)
            for jj in (1, 3):
                for ql in range(4):
                    out_proj_q(jj, ql)
            for ql in range(4):
                out_proj_q(2, ql)

    nc.compile()
    return nc


_NC_CACHE = None


def _get_nc():
    global _NC_CACHE
    if _NC_CACHE is None:
        _NC_CACHE = build_nc()
    return _NC_CACHE


def make_in_maps(x, Wq, Wk, Wv, Wo):
    bf16 = ml_dtypes.bfloat16
    x = np.asarray(x, dtype=np.float32)
    ws = [np.asarray(w, dtype=np.float32) for w in (Wq, Wk, Wv, Wo)]
    in_maps = []
    for c in range(N_CORES):
        b, g = c // 4, c % 4
        sl = slice(COC * g, COC * g + COC)
        in_maps.append(
            {
                "xT": np.ascontiguousarray(x[b].T).astype(bf16),
                "wqT": np.ascontiguousarray(ws[0][sl, :].T).astype(bf16),
                "wkT": np.ascontiguousarray(ws[1][sl, :].T).astype(bf16),
                "wvT": np.ascontiguousarray(ws[2][sl, :].T).astype(bf16),
                "woT": np.ascontiguousarray(ws[3][sl, :].T).astype(bf16),
            }
        )
    return in_maps


def assemble(results):
    out = np.empty((B, T, C), dtype=np.float32)
    for c in range(N_CORES):
        b, g = c // 4, c % 4
        out[b, :, COC * g : COC * g + COC] = results[c]["out"]
    return out


def kernel(x, Wq, Wk, Wv, Wo):
    nc = _get_nc()
    in_maps = make_in_maps(x, Wq, Wk, Wv, Wo)
    res = run_bass_kernel_spmd(nc, in_maps, list(range(N_CORES)))
    return assemble(res.results)


if __name__ == "__main__":
    rng = np.random.default_rng(0)
    x = rng.standard_normal((B, T, C), dtype=np.float32)
    s = 1.0 / np.sqrt(C)
    ws = [
        rng.uniform(-s, s, size=(C, C)).astype(np.float32) for _ in range(4)
    ]
    out = kernel(x, *ws)
    print("kernel ran; out", out.shape, out.dtype)

